# revision 12
# baseline (speedup 1.0000x reference)
"""Trainium2 Bass kernel for nn_DC_CRD_85779086836063 (gnn_message_passing).

Reference math (B,C,H,W = 32,64,128,128):
    wvec = mean(x, (2,3))                          # [B, C]
    diff = wvec[:,:,None] - wvec[:,None,:]         # [B, C, C]
    e = exp(-diff); T = |1 - e/(1+e)| - 1          # = sigmoid(diff) - 1
    A = 0.5*(T + T^T) * theta                      # sigmoid(d)+sigmoid(-d) = 1
                                                   # => T + T^T = -1 (exactly)
                                                   # => A = -0.5 * theta  (data-independent)
    H = relu(A @ x_flat)                           # [B, C, HW]
    out = (W_lin @ H)^T + b_lin  reshaped raw [HW,C] -> [C,H,W]

So per batch: out[b] (as [HW, C]) = (W_lin @ relu(-0.5 theta @ x[b]))^T + b_lin.

Sharding: pure data parallel, batch dim 32 -> 4 per core across 8 cores;
theta/W_lin/b_lin replicated.

Per-core dataflow (2-batch packing to fill 128 partitions, C=64). The
output transpose is FUSED into the second matmul: for a 128-column block
of pixels, out[n, (b,c')] = h_block^T @ blockdiag(W_lin^T, W_lin^T) with
the h block as the stationary operand — no separate transpose pass, no
separate bias pass (bias rides the PSUM->SBUF move on DVE).

    Ablk = blockdiag(-0.5 theta^T, -0.5 theta^T)   [128,128]  (lhsT of mm1)
    Wblk = blockdiag(W_lin^T, W_lin^T)             [128,128]  (rhs of mm2)
    per chunk (2048 px):
      one 1 MiB DMA loads x2 = [x[b0]; x[b1]] stacked [128, 2048] (f32r bits)
      per 512-col sub s:
        ps1 = Ablk.T @ x2_s                (PE)
        relu-scatter ps1 -> h              (ACT)  h layout [p, (j:16, q:128)]
                                           so h block j holds pixels n = 16q+j
      per PSUM tile t (4 j-blocks):
        ps3[:, jj*128:...] = h_j^T @ Wblk  (PE)   -> [q, (j4, 2b, c)]
        o[b, 4t+jj, c] = ps3 + bias        (DVE)  PSUM->SBUF, bias fused
      two 512 KiB DMAs store o -> out[b0/b1]; partition q owns 16
      consecutive DRAM rows (4 KiB contiguous runs).

Variants (BASS_VARIANT): "f32r" mm1+mm2 in float32r (x loaded as raw f32
                                bits reinterpreted f32r; ~3e-4 rel err)
                         "bf16" mm2 operands (h, Wblk) in bfloat16
"""

import os
import sys

sys.path.insert(0, "/opt/trn_rl_repo")

import numpy as np

import concourse.bacc as bacc
import concourse.mybir as mybir
from concourse import tile
from concourse.bass_utils import run_bass_kernel_spmd
from concourse.masks import make_identity

dt = mybir.dt
AF = mybir.ActivationFunctionType

B, C, H, W = 32, 64, 128, 128
HW = H * W
NCORES = 8
BL = B // NCORES  # batches per core
PAIRS = BL // 2

DMACHUNK = 2048  # pixels per chunk (1 MiB load)
SUB = 512  # cols per matmul / PSUM bank
NSUB = DMACHUNK // SUB
JB = DMACHUNK // 128  # 128-px transpose blocks per chunk (16)

VARIANT = os.environ.get("BASS_VARIANT", "f32r")


def _build(variant: str):
    d1 = dt.float32r  # mm1 operands (ablk, x)
    d2 = dt.bfloat16 if variant == "bf16" else dt.float32r  # mm2 operands

    nc = bacc.Bacc("TRN2", target_bir_lowering=False, debug=False)

    # x is declared float32r: same 4-byte layout as the f32 input, read
    # directly by the f32r matmul path with no cast pass.
    x_d = nc.dram_tensor("x", [BL, C, HW], d1, kind="ExternalInput")
    th_d = nc.dram_tensor("theta", [C, C], dt.float32, kind="ExternalInput")
    wl_d = nc.dram_tensor("W_lin", [C, C], dt.float32, kind="ExternalInput")
    bl_d = nc.dram_tensor("b_lin", [C], dt.float32, kind="ExternalInput")
    out_d = nc.dram_tensor("out", [BL, HW, C], dt.float32, kind="ExternalOutput")

    with tile.TileContext(nc) as tc:
        with (
            tc.tile_pool(name="const", bufs=1) as const,
            tc.tile_pool(name="xp", bufs=6) as xp,
            tc.tile_pool(name="hp", bufs=3) as hp,
            tc.tile_pool(name="op", bufs=3) as op_,
        ):
            # ---------------- constants ----------------
            psc_cm = tc.tile_pool(name="psc", bufs=1, space="PSUM")
            psc = psc_cm.__enter__()
            ident = const.tile([128, 128], dt.float32, tag="ident")
            make_identity(nc, ident[:])

            # block-diag(theta, theta) and block-diag(W_lin, W_lin) in SBUF
            thb = const.tile([128, 128], dt.float32, tag="thb")
            wlb = const.tile([128, 128], dt.float32, tag="wlb")
            # const DMAs ride the scalar HWDGE queue so the sync queue can
            # start streaming x from t=0. Memsets only touch the
            # off-diagonal quadrants so the diagonal-block DMAs don't wait
            # on them.
            nc.gpsimd.memset(thb[0:64, 64:128], 0.0)
            nc.gpsimd.memset(thb[64:128, 0:64], 0.0)
            nc.gpsimd.memset(wlb[0:64, 64:128], 0.0)
            nc.gpsimd.memset(wlb[64:128, 0:64], 0.0)
            nc.scalar.dma_start(thb[0:64, 0:64], th_d[:])
            nc.scalar.dma_start(thb[64:128, 64:128], th_d[:])
            nc.scalar.dma_start(wlb[0:64, 0:64], wl_d[:])
            nc.scalar.dma_start(wlb[64:128, 64:128], wl_d[:])

            # transpose on PE: psT = blockdiag(theta^T, theta^T), etc.
            psT = psc.tile([128, 512], dt.float32, tag="psT")
            nc.tensor.transpose(psT[:, 0:128], thb[:], ident[:])
            nc.tensor.transpose(psT[:, 128:256], wlb[:], ident[:])

            # Ablk = -0.5 * blockdiag(theta^T, theta^T)  (lhsT of mm1)
            ablk = const.tile([128, 128], d1, tag="ablk")
            nc.scalar.activation(ablk[:], psT[:, 0:128], AF.Copy, scale=-0.5)
            # Wblk = blockdiag(W_lin^T, W_lin^T)  (rhs/moving of mm2)
            wblk = const.tile([128, 128], d2, tag="wblk")
            nc.vector.tensor_copy(wblk[:], psT[:, 128:256])

            # bias broadcast to all 128 partitions via a K=1 matmul, then
            # tiled to [128, (2b, j4, c)] so the DVE add can read it with
            # any (b, j, c) view.
            ones = const.tile([1, 128], dt.float32, tag="ones")
            nc.gpsimd.memset(ones[:], 1.0)
            brow = const.tile([1, 64], dt.float32, tag="brow")
            nc.scalar.dma_start(brow[:], bl_d[:].rearrange("(one c) -> one c", one=1))
            psB = psc.tile([128, 64], dt.float32, tag="psB")
            nc.tensor.matmul(psB[:], ones[:], brow[:], start=True, stop=True)
            bias_bc = const.tile([128, 512], dt.float32, tag="bias_bc")
            for k in range(8):
                nc.vector.tensor_copy(bias_bc[:, 64 * k : 64 * (k + 1)], psB[:])

            psc_cm.__exit__(None, None, None)
            ps1p_cm = tc.tile_pool(name="ps1p", bufs=3, space="PSUM")
            ps3p_cm = tc.tile_pool(name="ps3p", bufs=4, space="PSUM")
            ps1p = ps1p_cm.__enter__()
            ps3p = ps3p_cm.__enter__()

            xsrc = x_d[:].rearrange("b c n -> (b c) n")
            biasv = bias_bc[:].rearrange("p (b j c) -> p b j c", b=2, j=4)

            # one-chunk software pipeline: stage A (load, mm1, relu) for
            # chunk i runs while stage B (mm2', bias-add, store) drains
            # chunk i-1 — keeps PE from stalling on the relu chain.
            NCH = PAIRS * (HW // DMACHUNK)
            state = {}
            for i in range(NCH + 1):
                if i < NCH:
                    pair, ci = divmod(i, HW // DMACHUNK)
                    b0 = 2 * pair
                    n0 = ci * DMACHUNK
                    x2 = xp.tile([128, DMACHUNK], d1, tag="x2")
                    # two half-chunk loads -> 4 KiB descriptors, matching
                    # the store descriptors: DMA engines round-robin queues
                    # per descriptor, so equal sizes give loads and stores
                    # a 1:1 bandwidth split instead of 2:1, keeping stores
                    # from piling up into a long drain after the last load.
                    for lh in range(2):
                        nc.sync.dma_start(
                            x2[:, lh * 1024 : (lh + 1) * 1024],
                            xsrc[
                                b0 * C : (b0 + 2) * C,
                                n0 + lh * 1024 : n0 + (lh + 1) * 1024,
                            ],
                        )
                    # h layout [p, (j:16, q:128)]: block j, col q holds
                    # pixel n = 16q + j of this chunk.
                    h = hp.tile([128, DMACHUNK], d2, tag="h")
                    hv = h[:].rearrange("p (j q) -> p j q", j=JB)
                    for s in range(NSUB):
                        ps1 = ps1p.tile([128, SUB], dt.float32, tag="ps1")
                        nc.tensor.matmul(
                            ps1[:],
                            ablk[:],
                            x2[:, s * SUB : (s + 1) * SUB],
                            start=True,
                            stop=True,
                        )
                        # relu + scatter: ps1 col m = 16*q2 + j goes to
                        # h[:, j, 32 s + q2]
                        ps1v = ps1[:].rearrange("p (q2 j) -> p j q2", j=JB)
                        nc.scalar.activation(
                            hv[:, :, 32 * s : 32 * (s + 1)], ps1v, AF.Relu
                        )
                    state[i] = (h, b0, n0)
                if i >= 1:
                    h, b0, n0 = state.pop(i - 1)
                    hv = h[:].rearrange("p (j q) -> p j q", j=JB)
                    # o layout [p, (b:2, j:16, c:64)]: per batch the free
                    # span (j, c) is 16 consecutive DRAM rows = 4 KiB.
                    o = op_.tile([128, 2 * DMACHUNK // 2], dt.float32, tag="o")
                    ov = o[:].rearrange("p (b j c) -> p b j c", b=2, j=JB)
                    for t in range(4):
                        ps3 = ps3p.tile([128, SUB], dt.float32, tag="ps3")
                        for jj in range(4):
                            nc.tensor.matmul(
                                ps3[:, jj * 128 : (jj + 1) * 128],
                                hv[:, 4 * t + jj, :],
                                wblk[:],
                                start=True,
                                stop=True,
                            )
                        # bias-add + PSUM->SBUF scatter (DVE)
                        ps3v = ps3[:].rearrange("p (j b c) -> p b j c", j=4, b=2)
                        nc.vector.tensor_add(
                            ov[:, :, 4 * t : 4 * (t + 1), :], ps3v, biasv
                        )
                    for bi in range(2):
                        dd = out_d[b0 + bi, n0 : n0 + DMACHUNK, :].rearrange(
                            "(q j) c -> q (j c)", q=128
                        )
                        nc.gpsimd.dma_start(
                            dd, o[:, bi * JB * C : (bi + 1) * JB * C]
                        )
            ps3p_cm.__exit__(None, None, None)
            ps1p_cm.__exit__(None, None, None)

    nc.compile()
    return nc


def _ensure_ntff_hook():
    """Register the axon NTFF profile hook (profiling only; best-effort).

    The agent image's ``antenv`` lacks ``axon_hooks``, so ``trace=True`` in
    ``run_bass_kernel_spmd`` would ImportError. Recreate the module with the
    same ctypes hook ``trn_agent_boot.trn_boot`` would have registered.
    """
    import contextlib
    import ctypes
    import types

    if "antenv.axon_hooks" in sys.modules:
        return
    so_path = "/opt/axon/libaxon_pjrt.so"
    try:
        lib = ctypes.CDLL(so_path)
        lib.axon_start_nrt_profile.argtypes = [
            ctypes.POINTER(ctypes.c_int64),
            ctypes.c_size_t,
        ]
        lib.axon_start_nrt_profile.restype = ctypes.c_int64
        lib.axon_stop_nrt_profile.argtypes = [ctypes.c_char_p]
        lib.axon_stop_nrt_profile.restype = ctypes.c_int64
    except (OSError, AttributeError):
        lib = None

    @contextlib.contextmanager
    def _hook(output_dir, device_ids):
        import jax

        jax.devices()
        if device_ids:
            ids = (ctypes.c_int64 * len(device_ids))(*device_ids)
            rc = lib.axon_start_nrt_profile(ids, len(device_ids))
        else:
            rc = lib.axon_start_nrt_profile(None, 0)
        if rc != 0:
            raise RuntimeError(f"axon_start_nrt_profile rc={rc}")
        try:
            yield
        finally:
            n = lib.axon_stop_nrt_profile(str(output_dir).encode())
            print(f"ntff profile: {n} file(s) written to {output_dir}")

    hook = _hook if lib is not None else None
    mod = types.ModuleType("antenv.axon_hooks")
    mod.get_axon_ntff_profile_hook = lambda: hook
    mod.set_axon_ntff_profile_hook = lambda h: None
    sys.modules["antenv.axon_hooks"] = mod


_NC_CACHE = {}


def _get_nc(variant: str):
    if variant not in _NC_CACHE:
        _NC_CACHE[variant] = _build(variant)
    return _NC_CACHE[variant]


def _run(inputs: dict, trace: bool = False, variant: str | None = None):
    variant = variant or VARIANT
    if trace:
        _ensure_ntff_hook()
    nc = _get_nc(variant)
    x = np.ascontiguousarray(inputs["x"], dtype=np.float32)
    theta = np.ascontiguousarray(inputs["theta"], dtype=np.float32)
    w_lin = np.ascontiguousarray(inputs["W_lin"], dtype=np.float32)
    b_lin = np.ascontiguousarray(inputs["b_lin"], dtype=np.float32)
    in_maps = [
        {
            "x": np.ascontiguousarray(x[i * BL : (i + 1) * BL].reshape(BL, C, HW)),
            "theta": theta,
            "W_lin": w_lin,
            "b_lin": b_lin,
        }
        for i in range(NCORES)
    ]
    # Occasionally the first execution of a freshly-loaded NEFF fails with
    # NRT_EXEC_UNIT_UNRECOVERABLE; a retry on the recovered device succeeds.
    import time

    last_err = None
    for attempt in range(4):
        try:
            res = run_bass_kernel_spmd(
                nc,
                in_maps,
                core_ids=list(range(NCORES)),
                trace=trace and attempt == 0,
            )
            break
        except Exception as e:  # noqa: BLE001
            last_err = e
            try:  # drop the (possibly dead) PJRT client; next call re-inits
                import jax

                jax.clear_caches()
                jax.extend.backend.clear_backends()
            except Exception:  # noqa: BLE001
                pass
            time.sleep(10 * (attempt + 1))
    else:
        raise last_err
    shards = [r["out"].reshape(BL, C, H, W) for r in res.results]
    return np.concatenate(shards, axis=0), res


def kernel(x, theta, W_lin, b_lin):
    out, _ = _run({"x": x, "theta": theta, "W_lin": W_lin, "b_lin": b_lin})
    return out
